# revision 8
# baseline (speedup 1.0000x reference)
"""Trainium2 Bass kernel for nn_BitwiseModule (scatter_memory).

Computation (per row of x [B, 512]):
  - active flags from cols 0..3 (op_and, op_or, op_xor, mark_ax; flag = v > 0.5)
  - a_lo/a_hi/b_lo/b_hi = argmax over cols [16:32),[32:48),[48:64),[64:80)
  - r = op(a, b) bitwise, op priority xor > or > and; nibble-wise:
      r_lo = op(a_lo, b_lo), r_hi = op(a_hi, b_hi)
  - out = x, plus 1.0 at cols 80+r_lo and 96+r_hi for active rows.

Sharding: pure data parallel over the batch dim across 8 cores.
"""

import os

import numpy as np

import bass_rust
import concourse.bass as bass
import concourse.mybir as mybir
from concourse.bass_utils import run_bass_kernel_spmd
from concourse.mybir import AluOpType
from concourse.tile import TileContext
from concourse.vector_clock import ScopedClock

B_FULL = 131072
D = 512
N_CORES = 8
R = B_FULL // N_CORES  # rows per core
P = 128

F32 = mybir.dt.float32
I32 = mybir.dt.int32


class SplitDrainTileContext(TileContext):
    """TileContext whose kernel-tail drain spreads its semaphore waits over
    several instructions: the bundled walrus codegen rejects instructions
    carrying more than two sync-wait commands."""

    def _drain_and_barrier(self, tick_clock, wait_clock):
        nc = self.nc
        drain_inst = nc.sync.drain()
        wait_clock.add_sem_waits(
            drain_inst.ins, ScopedClock({None: tick_clock.global_clock})
        )
        si = drain_inst.ins.sync_info
        if si is not None and len(si.on_wait) > 1:
            waits = list(si.on_wait)
            drain_inst.ins.sync_info = bass_rust.SyncInfo(
                on_wait=[waits[0]], on_update=list(si.on_update)
            )
            for w in waits[1:]:
                nop = nc.sync.nop()
                nop.ins.sync_info = bass_rust.SyncInfo(on_wait=[w], on_update=[])
        nc.all_engine_barrier()
        popped = nc._tile_sem_poison_stack.pop()
        assert popped is self._sem_poison
        nc.clear_and_free_semaphores(list(self.sems.allocated().values()))
        nc.all_engine_barrier()


def split_multi_waits(nc: bass.Bass, max_waits: int = 1) -> int:
    """The bundled walrus codegen rejects instructions with more than one or
    two sync-wait commands. Move surplus waits onto fresh same-engine NoOps
    inserted immediately before the offending instruction (waits-before is
    semantics-preserving)."""
    n_split = 0
    for f in nc.m.functions:
        for blk in f.blocks:
            insts = blk.instructions
            i = 0
            while i < len(insts):
                inst = insts[i]
                si = getattr(inst, "sync_info", None)
                if si is not None and len(si.on_wait) > max_waits:
                    waits = list(si.on_wait)
                    inst.sync_info = bass_rust.SyncInfo(
                        on_wait=waits[:max_waits], on_update=list(si.on_update)
                    )
                    nops = []
                    for k, w in enumerate(waits[max_waits:]):
                        nops.append(
                            mybir.InstNoOp(
                                name=f"{inst.name}-wsplit{k}",
                                engine=inst.engine,
                                bass_nofuse=True,
                                ins=[],
                                outs=[],
                                sync_info=mybir.SyncInfo(on_wait=[w], on_update=[]),
                            )
                        )
                    insts[i:i] = nops
                    i += len(nops)
                    n_split += 1
                i += 1
    return n_split


def build_kernel(rows: int = R, g: int = 8, bufs: int = 3) -> bass.Bass:
    """Build the per-core Bass program for a shard of `rows` rows.

    Layout: row = sg*(P*g) + p*g + j  (g consecutive rows per partition), so
    each partition's DMA chunk is g*2048 contiguous bytes.
    """
    assert rows % (P * g) == 0
    nsg = rows // (P * g)

    nc = bass.Bass(trn_type="TRN2")
    x = nc.dram_tensor("x", [rows, D], F32, kind="ExternalInput")
    y = nc.dram_tensor("y", [rows, D], F32, kind="ExternalOutput")
    x_v = x[:].rearrange("(s p j) d -> s p j d", p=P, j=g)
    y_v = y[:].rearrange("(s p j) d -> s p j d", p=P, j=g)

    with SplitDrainTileContext(nc) as tc:
        with (
            tc.tile_pool(name="const", bufs=1) as cpool,
            tc.tile_pool(name="x", bufs=bufs) as xpool,
            tc.tile_pool(name="mid", bufs=bufs) as mpool,
        ):
            # ---- constants ----
            iota_pb_i = cpool.tile([P, 16], I32)  # j + 256
            nc.gpsimd.iota(iota_pb_i[:], pattern=[[1, 16]], base=256, channel_multiplier=0)
            iota_pb = cpool.tile([P, 16], F32)
            nc.vector.tensor_copy(iota_pb[:], iota_pb_i[:])
            iota_lh = cpool.tile([P, 16], I32)  # 0..15
            nc.gpsimd.iota(iota_lh[:], pattern=[[1, 16]], base=0, channel_multiplier=0)
            neg1 = cpool.tile([P, 2 * g], I32)
            nc.vector.memset(neg1[:], -1)

            iota_pb_b = iota_pb[:].unsqueeze(1).broadcast_to((P, g * 4, 16))
            iota_lh_b = iota_lh[:].unsqueeze(1).broadcast_to((P, g * 2, 16))
            neg1_3 = neg1[:].rearrange("p (j h) -> p j h", j=g)

            for sg in range(nsg):
                X = xpool.tile([P, g * D], F32, name="X")
                X3 = X[:].rearrange("p (j d) -> p j d", j=g)
                nc.sync.dma_start(X3, x_v[sg])

                # compact copy of the 4 argmax fields so (group, field) merge
                # into one affine dim: F[p, k, v] with k = j*4 + f
                F = mpool.tile([P, g * 64], F32, name="F")
                F3 = F[:].rearrange("p (k v) -> p k v", v=16)
                nc.vector.tensor_copy(F3, X3[:, :, 16:80].rearrange("p j c -> p j c"))

                m = mpool.tile([P, g * 4], F32, name="m")
                nc.vector.tensor_reduce(
                    m[:], F3, axis=mybir.AxisListType.X, op=AluOpType.max
                )

                eq = mpool.tile([P, g * 64], F32, name="eq")
                eq3 = eq[:].rearrange("p (k v) -> p k v", v=16)
                m_b = m[:].unsqueeze(2).broadcast_to((P, g * 4, 16))
                nc.vector.tensor_tensor(eq3, F3, m_b, AluOpType.is_equal)
                # z = eq * (-256) + (iota + 256): j where eq (max), j+256 otherwise
                nc.vector.scalar_tensor_tensor(
                    eq3, eq3, -256.0, iota_pb_b, AluOpType.mult, AluOpType.add
                )
                idx = mpool.tile([P, g * 4], I32, name="idx")
                idx3 = idx[:].rearrange("p (j f) -> p j f", j=g)
                nc.vector.tensor_reduce(
                    idx[:], eq3, axis=mybir.AxisListType.X, op=AluOpType.min
                )

                # nibble-wise bitwise ops: fields [a_lo, a_hi] op [b_lo, b_hi]
                a2 = idx3[:, :, 0:2]
                b2 = idx3[:, :, 2:4]
                and_t = mpool.tile([P, g * 2], I32, name="and_t")
                and3 = and_t[:].rearrange("p (j h) -> p j h", j=g)
                nc.vector.tensor_tensor(and3, a2, b2, AluOpType.bitwise_and)
                or_t = mpool.tile([P, g * 2], I32, name="or_t")
                or3 = or_t[:].rearrange("p (j h) -> p j h", j=g)
                nc.vector.tensor_tensor(or3, a2, b2, AluOpType.bitwise_or)
                xor_t = mpool.tile([P, g * 2], I32, name="xor_t")
                xor3 = xor_t[:].rearrange("p (j h) -> p j h", j=g)
                nc.vector.tensor_tensor(xor3, a2, b2, AluOpType.bitwise_xor)

                # active flags, duplicated per (lo, hi) so masks are compact
                def flag_mask(col, op, tag):
                    t = mpool.tile([P, g * 2], I32, name=tag)
                    t3 = t[:].rearrange("p (j h) -> p j h", j=g)
                    src = X3[:, :, col : col + 1].broadcast_to((P, g, 2))
                    nc.vector.tensor_scalar(t3, src, 0.5, None, op)
                    return t3

                ga = flag_mask(0, AluOpType.is_gt, "ga")
                go = flag_mask(1, AluOpType.is_gt, "go")
                gx = flag_mask(2, AluOpType.is_gt, "gx")
                gm_n = flag_mask(3, AluOpType.is_le, "gm_n")

                # priority select: xor > or > and; -1 when inactive
                r = mpool.tile([P, g * 2], I32, name="r")
                r3 = r[:].rearrange("p (j h) -> p j h", j=g)
                nc.vector.tensor_copy(r3, neg1_3)
                nc.vector.copy_predicated(r3, ga, and3)
                nc.vector.copy_predicated(r3, go, or3)
                nc.vector.copy_predicated(r3, gx, xor3)
                nc.vector.copy_predicated(r3, gm_n, neg1_3)

                # one-hot delta and add into cols 80..112
                d = mpool.tile([P, g * 32], F32, name="d")
                d3h = d[:].rearrange("p (k v) -> p k v", v=16)
                r_b = r[:].unsqueeze(2).broadcast_to((P, g * 2, 16))
                nc.vector.tensor_tensor(d3h, iota_lh_b, r_b, AluOpType.is_equal)
                d3 = d[:].rearrange("p (j w) -> p j w", j=g)
                xmod = X3[:, :, 80:112]
                nc.vector.tensor_tensor(xmod, xmod, d3, AluOpType.add)

                nc.sync.dma_start(y_v[sg], X3)

    split_multi_waits(nc)
    return nc


_CACHED = {}


def _get_kernel(rows: int = R):
    key = rows
    if key not in _CACHED:
        _CACHED[key] = build_kernel(rows)
    return _CACHED[key]


def kernel(x: np.ndarray, _trace: bool = False):
    x = np.ascontiguousarray(np.asarray(x, dtype=np.float32))
    assert x.shape == (B_FULL, D), x.shape
    nc = _get_kernel(R)
    shards = [x[i * R : (i + 1) * R] for i in range(N_CORES)]
    in_maps = [{"x": s} for s in shards]
    res = run_bass_kernel_spmd(
        nc, in_maps, core_ids=list(range(N_CORES)), trace=_trace
    )
    out = np.concatenate([res.results[i]["y"] for i in range(N_CORES)], axis=0)
    if _trace:
        kernel._last_results = res
    return out


# revision 29
# speedup vs baseline: 1.3522x; 1.3522x over previous
"""Trainium2 Bass kernel for nn_BitwiseModule (scatter_memory).

Computation (per row of x [B, 512]):
  - active flags from cols 0..3 (op_and, op_or, op_xor, mark_ax; flag = v > 0.5)
  - a_lo/a_hi/b_lo/b_hi = argmax over cols [16:32),[32:48),[48:64),[64:80)
  - r = op(a, b) bitwise, op priority xor > or > and; nibble-wise:
      r_lo = op(a_lo, b_lo), r_hi = op(a_hi, b_hi)
  - out = x, plus 1.0 at cols 80+r_lo and 96+r_hi for active rows.

Sharding: pure data parallel over the batch dim across 8 cores.
"""

import os

import numpy as np

import bass_rust
import concourse.bass as bass
import concourse.mybir as mybir
from concourse.bass_utils import run_bass_kernel_spmd
from concourse.mybir import AluOpType
from concourse.tile import TileContext
from concourse.vector_clock import ScopedClock

B_FULL = 131072
D = 512
N_CORES = 8
R = B_FULL // N_CORES  # rows per core
P = 128

F32 = mybir.dt.float32
I32 = mybir.dt.int32


class SplitDrainTileContext(TileContext):
    """TileContext whose kernel-tail drain spreads its semaphore waits over
    several instructions: the bundled walrus codegen rejects instructions
    carrying more than two sync-wait commands."""

    def _drain_and_barrier(self, tick_clock, wait_clock):
        nc = self.nc
        drain_inst = nc.sync.drain()
        wait_clock.add_sem_waits(
            drain_inst.ins, ScopedClock({None: tick_clock.global_clock})
        )
        si = drain_inst.ins.sync_info
        if si is not None and len(si.on_wait) > 1:
            waits = list(si.on_wait)
            drain_inst.ins.sync_info = bass_rust.SyncInfo(
                on_wait=[waits[0]], on_update=list(si.on_update)
            )
            for w in waits[1:]:
                nop = nc.sync.nop()
                nop.ins.sync_info = bass_rust.SyncInfo(on_wait=[w], on_update=[])
        nc.all_engine_barrier()
        popped = nc._tile_sem_poison_stack.pop()
        assert popped is self._sem_poison
        nc.clear_and_free_semaphores(list(self.sems.allocated().values()))
        nc.all_engine_barrier()


def split_multi_waits(nc: bass.Bass, max_waits: int = 1) -> int:
    """The bundled walrus codegen rejects instructions with more than one or
    two sync-wait commands. Move surplus waits onto fresh same-engine NoOps
    inserted immediately before the offending instruction (waits-before is
    semantics-preserving)."""
    n_split = 0
    for f in nc.m.functions:
        for blk in f.blocks:
            insts = blk.instructions
            i = 0
            while i < len(insts):
                inst = insts[i]
                si = getattr(inst, "sync_info", None)
                if si is not None and len(si.on_wait) > max_waits:
                    waits = list(si.on_wait)
                    inst.sync_info = bass_rust.SyncInfo(
                        on_wait=waits[:max_waits], on_update=list(si.on_update)
                    )
                    nops = []
                    for k, w in enumerate(waits[max_waits:]):
                        nops.append(
                            mybir.InstNoOp(
                                name=f"{inst.name}-wsplit{k}",
                                engine=inst.engine,
                                bass_nofuse=True,
                                ins=[],
                                outs=[],
                                sync_info=mybir.SyncInfo(on_wait=[w], on_update=[]),
                            )
                        )
                    insts[i:i] = nops
                    i += len(nops)
                    n_split += 1
                i += 1
    return n_split


def build_kernel(
    rows: int = R,
    g: int = 8,
    bufs: int = 3,
    store_engine: str = "sync",
    cw: int = D,
    d2d_chunks: int = 16,
    offload: bool = False,
    mbufs: int | None = None,
) -> bass.Bass:
    """Build the per-core Bass program for a shard of `rows` rows.

    Layout: row = sg*(P*g) + p*g + j  (g consecutive rows per partition), so
    each partition's DMA chunk is g*cw*4 contiguous bytes.

    cw < D enables the split strategy: columns [0, cw) go through SBUF
    (compute + copy); columns [cw, D) are copied DRAM->DRAM on the scalar
    HWDGE ring, bypassing SBUF entirely. cw must be >= 112.
    """
    assert rows % (P * g) == 0
    assert cw >= 112
    assert rows % d2d_chunks == 0
    nsg = rows // (P * g)

    nc = bass.Bass(trn_type="TRN2")
    store_eng = {
        "sync": nc.sync,
        "scalar": nc.scalar,
        "alt": nc.sync,
        "paced": nc.sync,
    }[store_engine]
    x = nc.dram_tensor("x", [rows, D], F32, kind="ExternalInput")
    y = nc.dram_tensor("y", [rows, D], F32, kind="ExternalOutput")
    x_v = x[:].rearrange("(s p j) d -> s p j d", p=P, j=g)
    y_v = y[:].rearrange("(s p j) d -> s p j d", p=P, j=g)

    with SplitDrainTileContext(nc) as tc:
        with (
            tc.tile_pool(name="const", bufs=1) as cpool,
            tc.tile_pool(name="x", bufs=bufs) as xpool,
            tc.tile_pool(name="mid", bufs=mbufs or bufs) as mpool,
        ):
            # ---- constants ----
            iota_pb_i = cpool.tile([P, 16], I32)  # j + 256
            nc.gpsimd.iota(iota_pb_i[:], pattern=[[1, 16]], base=256, channel_multiplier=0)
            iota_pb = cpool.tile([P, 16], F32)
            nc.vector.tensor_copy(iota_pb[:], iota_pb_i[:])
            iota_lh = cpool.tile([P, 16], I32)  # 0..15
            nc.gpsimd.iota(iota_lh[:], pattern=[[1, 16]], base=0, channel_multiplier=0)
            neg1 = cpool.tile([P, 2 * g], I32)
            nc.vector.memset(neg1[:], -1)

            iota_pb_b = iota_pb[:].unsqueeze(1).broadcast_to((P, g * 4, 16))
            iota_lh_b = iota_lh[:].unsqueeze(1).broadcast_to((P, g * 2, 16))
            neg1_3 = neg1[:].rearrange("p (j h) -> p j h", j=g)

            # DRAM->DRAM copy of columns [cw, D) — never touches SBUF
            alt = store_engine == "alt"
            paced = store_engine == "paced"
            d2d_todo = []
            if cw < D:
                rc = rows // d2d_chunks
                for c in range(d2d_chunks):
                    src = x[c * rc : (c + 1) * rc, cw:D]
                    dst = y[c * rc : (c + 1) * rc, cw:D]
                    if paced:
                        d2d_todo.append((dst, src))
                    else:
                        eng = (nc.scalar if c % 2 else nc.sync) if alt else nc.scalar
                        eng.dma_start(dst, src)
            if paced:
                # prime the d2d stream with a couple of chunks
                for _ in range(min(2, len(d2d_todo))):
                    dst, src = d2d_todo.pop(0)
                    nc.scalar.dma_start(dst, src)
            pace_scratch = cpool.tile([P, 2], F32)

            for sg in range(nsg):
                load_eng = (nc.sync if sg % 2 else nc.scalar) if alt else nc.sync
                X = xpool.tile([P, g * cw], F32, name="X")
                X3 = X[:].rearrange("p (j d) -> p j d", j=g)
                load_eng.dma_start(X3, x_v[sg][:, :, 0:cw])

                aux = nc.gpsimd if offload else nc.vector

                # compact copy of the 4 argmax fields so (group, field) merge
                # into one affine dim: F[p, k, v] with k = j*4 + f
                F = mpool.tile([P, g * 64], F32, name="F")
                F3 = F[:].rearrange("p (k v) -> p k v", v=16)
                aux.tensor_copy(F3, X3[:, :, 16:80].rearrange("p j c -> p j c"))

                m = mpool.tile([P, g * 4], F32, name="m")
                nc.vector.tensor_reduce(
                    m[:], F3, axis=mybir.AxisListType.X, op=AluOpType.max
                )

                eq = mpool.tile([P, g * 64], F32, name="eq")
                eq3 = eq[:].rearrange("p (k v) -> p k v", v=16)
                m_b = m[:].unsqueeze(2).broadcast_to((P, g * 4, 16))
                nc.vector.tensor_tensor(eq3, F3, m_b, AluOpType.is_equal)
                # z = eq * (-256) + (iota + 256): j where eq (max), j+256 otherwise
                nc.vector.scalar_tensor_tensor(
                    eq3, eq3, -256.0, iota_pb_b, AluOpType.mult, AluOpType.add
                )
                idx = mpool.tile([P, g * 4], I32, name="idx")
                idx3 = idx[:].rearrange("p (j f) -> p j f", j=g)
                nc.vector.tensor_reduce(
                    idx[:], eq3, axis=mybir.AxisListType.X, op=AluOpType.min
                )

                # nibble-wise bitwise ops: fields [a_lo, a_hi] op [b_lo, b_hi]
                a2 = idx3[:, :, 0:2]
                b2 = idx3[:, :, 2:4]
                and_t = mpool.tile([P, g * 2], I32, name="and_t")
                and3 = and_t[:].rearrange("p (j h) -> p j h", j=g)
                nc.vector.tensor_tensor(and3, a2, b2, AluOpType.bitwise_and)
                or_t = mpool.tile([P, g * 2], I32, name="or_t")
                or3 = or_t[:].rearrange("p (j h) -> p j h", j=g)
                nc.vector.tensor_tensor(or3, a2, b2, AluOpType.bitwise_or)
                xor_t = mpool.tile([P, g * 2], I32, name="xor_t")
                xor3 = xor_t[:].rearrange("p (j h) -> p j h", j=g)
                nc.vector.tensor_tensor(xor3, a2, b2, AluOpType.bitwise_xor)

                # active flags, duplicated per (lo, hi) so masks are compact
                def flag_mask(col, op, tag):
                    t = mpool.tile([P, g * 2], I32, name=tag)
                    t3 = t[:].rearrange("p (j h) -> p j h", j=g)
                    src = X3[:, :, col : col + 1].broadcast_to((P, g, 2))
                    aux.tensor_scalar(t3, src, 0.5, None, op)
                    return t3

                ga = flag_mask(0, AluOpType.is_gt, "ga")
                go = flag_mask(1, AluOpType.is_gt, "go")
                gx = flag_mask(2, AluOpType.is_gt, "gx")
                gm_n = flag_mask(3, AluOpType.is_le, "gm_n")

                # priority select: xor > or > and; -1 when inactive
                r = mpool.tile([P, g * 2], I32, name="r")
                r3 = r[:].rearrange("p (j h) -> p j h", j=g)
                aux.memset(r[:], -1)
                nc.vector.copy_predicated(r3, ga, and3)
                nc.vector.copy_predicated(r3, go, or3)
                nc.vector.copy_predicated(r3, gx, xor3)
                nc.vector.copy_predicated(r3, gm_n, neg1_3)

                # one-hot delta and add into cols 80..112
                d = mpool.tile([P, g * 32], F32, name="d")
                d3h = d[:].rearrange("p (k v) -> p k v", v=16)
                r_b = r[:].unsqueeze(2).broadcast_to((P, g * 2, 16))
                nc.vector.tensor_tensor(d3h, iota_lh_b, r_b, AluOpType.is_equal)
                d3 = d[:].rearrange("p (j w) -> p j w", j=g)
                xmod = X3[:, :, 80:112]
                nc.vector.tensor_tensor(xmod, xmod, d3, AluOpType.add)

                seng = (nc.scalar if sg % 2 else nc.sync) if alt else store_eng
                seng.dma_start(y_v[sg][:, :, 0:cw], X3)

                if paced and d2d_todo:
                    # ACT-ring stub depending on this supergroup's compute
                    # throttles the next d2d chunk's descriptor generation,
                    # so the d2d stream can't starve the SBUF path.
                    nc.scalar.copy(pace_scratch[:, 0:2], d[:, 0:2])
                    n_rel = max(1, len(d2d_todo) // max(1, nsg - sg - 1) if nsg - sg - 1 else len(d2d_todo))
                    for _ in range(n_rel):
                        if d2d_todo:
                            dst, src = d2d_todo.pop(0)
                            nc.scalar.dma_start(dst, src)
            for dst, src in d2d_todo:
                nc.scalar.dma_start(dst, src)

    split_multi_waits(nc)
    return nc


_CACHED = {}


def _get_kernel(rows: int = R):
    key = rows
    if key not in _CACHED:
        _CACHED[key] = build_kernel(
            rows, g=16, bufs=8, store_engine="sync", cw=128, d2d_chunks=16
        )
    return _CACHED[key]


def kernel(x: np.ndarray, _trace: bool = False):
    x = np.ascontiguousarray(np.asarray(x, dtype=np.float32))
    assert x.shape == (B_FULL, D), x.shape
    nc = _get_kernel(R)
    shards = [x[i * R : (i + 1) * R] for i in range(N_CORES)]
    in_maps = [{"x": s} for s in shards]
    res = run_bass_kernel_spmd(
        nc, in_maps, core_ids=list(range(N_CORES)), trace=_trace
    )
    out = np.concatenate([res.results[i]["y"] for i in range(N_CORES)], axis=0)
    if _trace:
        kernel._last_results = res
    return out
